# revision 7
# baseline (speedup 1.0000x reference)
"""Trainium2 Bass kernel for single-head attention (B=8, S=2048, DIN=768, DOUT=64).

Strategy: data parallel — one batch element per NeuronCore (8 cores).
Per core, attention runs in transposed-score layout (k on partitions, q on
free dim), Scalar-engine-paced at ~1.11us per [128,1024] exp:

  warmup    ~6us of scratch matmuls at kernel start so the PE HAM clock
            gate latches 8/8 (2.4 GHz) while input DMAs stream
  qk proj   fp8e4 DoubleRow matmuls (x and Wqk prepacked on host into
            DR layout over din pairs; moving pairs byte-interleaved so the
            PE streams 2 fp8/cycle) -> PSUM [q|k, s-cols]
  repack    DVE copies PSUM quarters into q_dr (interleaved moving layout)
            and k_dr (pair-planar stationary layout), bias added
  mask      additive -2048 bias, fp8e5, applied by a DoubleRow PE matmul
            (identity stationary) accumulating into the score PSUM --
            exp(scale*(s-2048)) == 0, so no vector-engine mask multiply
  scores    fp8e4 DoubleRow matmul (2 cols/cycle), on top of the mask bias
  exp       ScalarE activation, PSUM fp32 -> SBUF bf16 (the bottleneck:
            32 x [128,1024] = ~35.6us busy)
  ctx       bf16 matmul, v65 = [v | 1] stationary (row 64 = softmax denom)
  epilogue  PE transposes + reciprocal + scale, interleaved into pass 1

Loop is q-half-major (2 passes x 16 k-tiles) so ctx PSUM is 2 banks and the
score PSUM triple-buffers (3x2 banks): 3*2 + 2 = 8 banks, keeping ScalarE
gap-free. v projections (bf16; fp8 v would cost ~2.4% output error) are
interleaved into pass 0 so their LDWEIGHTS hide under main-loop matmuls.
"""

import math
import sys
from contextlib import ExitStack

import numpy as np

sys.path.insert(0, "/opt/trn_rl_repo")

import ml_dtypes  # noqa: E402

import concourse.bass as bass  # noqa: E402
import concourse.tile as tile  # noqa: E402
from concourse import bacc, mybir  # noqa: E402
from concourse.bass import ds  # noqa: E402
from concourse.bass_utils import run_bass_kernel_spmd  # noqa: E402
from concourse.masks import make_identity  # noqa: E402

B, S, DIN, DOUT = 8, 2048, 768, 64
P = 128
NJ = 3  # din chunk-pairs for the DR qk projection
NCH = 6  # din chunks for the bf16 v projection
KT = S // P  # 16 key tiles
NB = 4  # qk projection column blocks of 512
NS = 512  # matmul moving free dim (one PSUM bank fp32)
H = 2  # q halves (passes)
HQ = S // H  # 1024
NWARM = 14  # scratch matmuls to latch the PE HAM gate at 8/8

F32 = mybir.dt.float32
BF16 = mybir.dt.bfloat16
FP8E4 = mybir.dt.float8e4
FP8E5 = mybir.dt.float8e5
DR = mybir.MatmulPerfMode.DoubleRow

_NC_CACHE = None


def build_nc():
    nc = bacc.Bacc("TRN2", target_bir_lowering=False, debug=False)

    xdr = nc.declare_dram_parameter("xdr", [NJ, NB, P, NS // 16, 2, 16], FP8E4, isOutput=False)
    xbf = nc.declare_dram_parameter("xbf", [NCH, P, S], BF16, isOutput=False)
    mb = nc.declare_dram_parameter("mb", [KT, 64, S // 16, 2, 16], FP8E5, isOutput=False)
    wqk = nc.declare_dram_parameter("wqk", [NJ, P, 2, P], FP8E4, isOutput=False)
    wv = nc.declare_dram_parameter("wv", [NCH, P, DOUT], BF16, isOutput=False)
    idr = nc.declare_dram_parameter("idr", [64, 2, P], FP8E5, isOutput=False)
    bqk = nc.declare_dram_parameter("bqk", [P, 1], F32, isOutput=False)
    out = nc.declare_dram_parameter("out", [S, DOUT], F32, isOutput=True)

    inv_sqrt_s = float(1.0 / math.sqrt(S))

    with tile.TileContext(nc) as tc, ExitStack() as ctx:
        singles = ctx.enter_context(tc.tile_pool(name="singles", bufs=1))
        epool = ctx.enter_context(tc.tile_pool(name="epool", bufs=3))
        opool = ctx.enter_context(tc.tile_pool(name="opool", bufs=4))

        # ---- constants / weights (small DMAs first)
        wqk_sb = singles.tile([P, NJ, 2, P], FP8E4)
        nc.sync.dma_start(out=wqk_sb, in_=wqk.rearrange("j p g m -> p j g m"))
        idr_sb = singles.tile([64, 2, P], FP8E5)
        nc.sync.dma_start(out=idr_sb, in_=idr[:, :, :])
        bqk_sb = singles.tile([P, 1], F32)
        nc.sync.dma_start(out=bqk_sb, in_=bqk[:, :])
        wv_sb = singles.tile([P, NCH, DOUT], BF16)
        nc.sync.dma_start(out=wv_sb, in_=wv.rearrange("c p m -> p c m"))

        # ---- big inputs, in consumption-priority order
        xdr_sb = singles.tile([P, NJ, NB, NS // 16, 2, 16], FP8E4)
        mb_sb = singles.tile([64, KT, S // 16, 2, 16], FP8E5)
        xbf_sb = singles.tile([P, NCH, S], BF16)
        for blk in range(2):
            for j in range(NJ):
                nc.sync.dma_start(
                    out=xdr_sb[:, j, blk, :, :, :], in_=xdr[j, blk, :, :, :, :]
                )
        for t in range(2):
            nc.sync.dma_start(out=mb_sb[:, t, :, :, :], in_=mb[t, :, :, :, :])
        for c in range(NCH):
            nc.sync.dma_start(out=xbf_sb[:, c, :], in_=xbf[c, :, :])
        for blk in range(2, NB):
            for j in range(NJ):
                nc.sync.dma_start(
                    out=xdr_sb[:, j, blk, :, :, :], in_=xdr[j, blk, :, :, :, :]
                )
        for t in range(2, KT):
            nc.sync.dma_start(out=mb_sb[:, t, :, :, :], in_=mb[t, :, :, :, :])

        ident = singles.tile([P, P], F32)
        make_identity(nc, ident)

        # ---- v with a ones column: [s(128 part), ktile, 65] bf16
        v65_sb = singles.tile([P, KT, DOUT + 1], BF16)
        nc.gpsimd.memset(v65_sb, 1.0)

        warm_sb = singles.tile([P, NS], BF16)
        nc.gpsimd.memset(warm_sb, 0.0)

        q_dr = singles.tile([32, S // 16, 2, 16], FP8E4)
        k_dr = singles.tile([32, 2, S], FP8E4)
        ctxT_sb = singles.tile([DOUT + 1, S], F32)

        with (
            tc.tile_pool(name="psS", bufs=3, space="PSUM") as psS,
            tc.tile_pool(name="psC", bufs=1, space="PSUM") as psC,
        ):
            # ---- HAM warmup: dense scratch matmuls while input DMAs stream
            wps = psS.tile([P, HQ], F32, tag="big")
            for w in range(NWARM):
                nc.tensor.matmul(
                    wps[:, 0:NS],
                    lhsT=warm_sb[:, 0:P],
                    rhs=warm_sb[:, 0:NS],
                    start=(w == 0),
                    stop=(w == NWARM - 1),
                )

            def vproj(t):
                v_ps = psS.tile([P, HQ], F32, tag="big")
                for c in range(NCH):
                    nc.tensor.matmul(
                        v_ps[:, 0:DOUT],
                        lhsT=xbf_sb[:, c, ds(t * P, P)],
                        rhs=wv_sb[:, c, :],
                        start=(c == 0),
                        stop=(c == NCH - 1),
                    )
                nc.vector.tensor_copy(v65_sb[:, t, 0:DOUT], v_ps[:, 0:DOUT])

            # ---- qk projection (fp8 DR) + repack to q_dr/k_dr
            for blk in range(NB):
                qk_ps = psS.tile([P, HQ], F32, tag="big")
                for j in range(NJ):
                    nc.tensor.matmul(
                        qk_ps[:, 0:NS],
                        lhsT=wqk_sb[:, j, :, :],
                        rhs=xdr_sb[:, j, blk, :, :, :].rearrange("p c g m -> p g c m"),
                        start=(j == 0),
                        stop=(j == NJ - 1),
                        perf_mode=DR,
                    )
                cols = ds(blk * NS, NS)
                c16 = ds(blk * (NS // 16), NS // 16)
                nc.vector.tensor_scalar_add(
                    q_dr[:, c16, 0, :],
                    qk_ps[0:32, 0:NS].rearrange("p (c m) -> p c m", m=16),
                    bqk_sb[0:32],
                )
                nc.vector.tensor_scalar_add(
                    q_dr[:, c16, 1, :],
                    qk_ps[32:64, 0:NS].rearrange("p (c m) -> p c m", m=16),
                    bqk_sb[32:64],
                )
                nc.vector.tensor_scalar_add(
                    k_dr[:, 0, cols], qk_ps[64:96, 0:NS], bqk_sb[64:96]
                )
                nc.vector.tensor_scalar_add(
                    k_dr[:, 1, cols], qk_ps[96:128, 0:NS], bqk_sb[96:128]
                )
                if blk == 1:
                    vproj(0)

            # ---- epilogue worker: transpose back, normalize, stage stores
            ostage = {}

            def epilogue_tile(h, qt):
                tr = psS.tile([P, HQ], F32, tag="big")
                nc.tensor.transpose(
                    tr[:, 0 : DOUT + 1],
                    ctxT_sb[:, ds(h * HQ + qt * P, P)],
                    ident[0 : DOUT + 1, 0 : DOUT + 1],
                )
                rc = opool.tile([P, 1], F32, tag="rc")
                nc.vector.reciprocal(rc, tr[:, DOUT : DOUT + 1])
                g, gi = qt // 4, qt % 4
                if gi == 0:
                    ostage[(h, g)] = opool.tile(
                        [P, 4, DOUT], F32, tag="ostage", name=f"ostage_{h}_{g}"
                    )
                nc.vector.tensor_scalar_mul(
                    ostage[(h, g)][:, gi, :], tr[:, 0:DOUT], rc
                )
                if gi == 3:
                    nc.sync.dma_start(
                        out=out[ds(h * HQ + g * 4 * P, 4 * P), :].rearrange(
                            "(t p) m -> p t m", p=P
                        ),
                        in_=ostage.pop((h, g)),
                    )

            # ---- main loop: q-half-major, 16 k-tiles inside
            ep_queue = []  # deferred epilogue work, interleaved into pass 1
            for h in range(H):
                ctx_ps = psC.tile([DOUT + 1, HQ], F32)
                for t in range(KT):
                    sc = psS.tile([P, HQ], F32, tag="big")
                    for n in range(HQ // NS):
                        qc16 = ds((h * HQ + n * NS) // 16, NS // 16)
                        nc.tensor.matmul(
                            sc[:, ds(n * NS, NS)],
                            lhsT=idr_sb,
                            rhs=mb_sb[:, t, qc16, :, :].rearrange("p c g m -> p g c m"),
                            start=True,
                            stop=False,
                            perf_mode=DR,
                        )
                        nc.tensor.matmul(
                            sc[:, ds(n * NS, NS)],
                            lhsT=k_dr[:, :, ds(t * P, P)],
                            rhs=q_dr[:, qc16, :, :].rearrange("p c g m -> p g c m"),
                            start=False,
                            stop=True,
                            perf_mode=DR,
                        )
                    ex = epool.tile([P, HQ], BF16, tag="exp")
                    nc.scalar.activation(
                        out=ex,
                        in_=sc,
                        func=mybir.ActivationFunctionType.Exp,
                        scale=inv_sqrt_s,
                    )
                    for n in range(HQ // NS):
                        nc.tensor.matmul(
                            ctx_ps[:, ds(n * NS, NS)],
                            lhsT=v65_sb[:, t, :],
                            rhs=ex[:, ds(n * NS, NS)],
                            start=(t == 0),
                            stop=(t == KT - 1),
                        )
                    if h == 0 and t < KT - 1:
                        vproj(t + 1)
                    # spread pass-0 epilogue work into pass 1
                    if ep_queue and t >= 1:
                        epilogue_tile(*ep_queue.pop(0))

                nc.vector.tensor_copy(ctxT_sb[:, ds(h * HQ, HQ)], ctx_ps)
                ep_queue.extend((h, qt) for qt in range(HQ // P))

            while ep_queue:
                epilogue_tile(*ep_queue.pop(0))

    nc.finalize()
    return nc


def _get_nc():
    global _NC_CACHE
    if _NC_CACHE is None:
        _NC_CACHE = build_nc()
    return _NC_CACHE


def kernel(**inputs):
    x = np.asarray(inputs["input_tensor"], dtype=np.float32)  # [B, S, DIN]
    mask = np.asarray(inputs["attention_mask"])  # [B, S, S] bool
    Wq = np.asarray(inputs["Wq"], dtype=np.float32)
    Wk = np.asarray(inputs["Wk"], dtype=np.float32)
    Wv = np.asarray(inputs["Wv"], dtype=np.float32)
    bq = np.asarray(inputs["bq"], dtype=np.float32)
    bk = np.asarray(inputs["bk"], dtype=np.float32)
    bv = np.asarray(inputs["bv"], dtype=np.float32)

    # Wqk in DR layout over din pairs: [j, p, g, m], din = 384g + 128j + p
    Wqk = np.concatenate([Wq, Wk], axis=1)  # [768, 128]
    wqk_h = np.ascontiguousarray(
        Wqk.reshape(2, NJ, P, P).transpose(1, 2, 0, 3)
    ).astype(ml_dtypes.float8_e4m3)
    wv_h = np.ascontiguousarray(Wv.reshape(NCH, P, DOUT)).astype(ml_dtypes.bfloat16)
    bqk_h = np.ascontiguousarray(np.concatenate([bq, bk]).reshape(P, 1))

    # DR identity for the additive mask matmul: idr[p, g, 64g+p] = 1
    idr_h = np.zeros((64, 2, P), dtype=np.float32)
    pp = np.arange(64)
    idr_h[pp, 0, pp] = 1.0
    idr_h[pp, 1, 64 + pp] = 1.0
    idr_h = idr_h.astype(ml_dtypes.float8_e5m2)

    in_maps = []
    for b in range(B):
        xT = np.ascontiguousarray(x[b].T)  # [DIN, S] fp32
        # x in DR fp8, pair-interleaved, block-major:
        # [j, blk, p, s', g], din = 384g + 128j + p
        xdr_h = np.ascontiguousarray(
            xT.reshape(2, NJ, P, NB, NS // 16, 16).transpose(1, 3, 2, 4, 0, 5)
        ).astype(ml_dtypes.float8_e4m3)
        xbf_h = np.ascontiguousarray(xT.reshape(NCH, P, S)).astype(ml_dtypes.bfloat16)
        # mask bias in DR fp8e5, pair-interleaved: [t, p, q, g], key = 128t+64g+p
        maskT = mask[b].T  # [k, q]
        mb_h = np.ascontiguousarray(
            np.where(maskT, np.float32(-2048.0), np.float32(0.0))
            .reshape(KT, 2, 64, S // 16, 16)
            .transpose(0, 2, 3, 1, 4)
        ).astype(ml_dtypes.float8_e5m2)
        in_maps.append(
            {
                "xdr": xdr_h,
                "xbf": xbf_h,
                "mb": mb_h,
                "wqk": wqk_h,
                "wv": wv_h,
                "idr": idr_h,
                "bqk": bqk_h,
            }
        )

    nc = _get_nc()
    res = run_bass_kernel_spmd(nc, in_maps, core_ids=list(range(B)))
    out = np.stack([np.asarray(res.results[b]["out"], np.float32) for b in range(B)])
    out = out + bv[None, None, :]
    return out.astype(np.float32)


# revision 9
# speedup vs baseline: 1.6924x; 1.6924x over previous
"""Trainium2 Bass kernel for single-head attention (B=8, S=2048, DIN=768, DOUT=64).

Strategy: data parallel — one batch element per NeuronCore (8 cores).
Per core, attention runs in transposed-score layout (k on partitions, q on
free dim), ScalarE-paced at ~1.11us per [128,1024] exp. All math is bf16
with fp32 PSUM accumulation (fp8/DoubleRow measures 2 cyc/col on this HW —
no gain — so bf16 at 1 col/cycle is optimal).

  warmup    scratch matmuls bridge every prologue gap so the PE HAM clock
            gate latches 8/8 (2.4 GHz) and the main loop never idles long
            enough (>3.4us) to drop back to 1.2 GHz
  qk proj   [Wq|Wk] combined: 6 chunk matmuls per 512-col block ->
            PSUM [q|k, cols]; DVE splits into qT/kT (bias added)
  scores    kT-tile stationary [64,128], qT moving -> sc[k,q] PSUM
  exp       ScalarE activation, PSUM fp32 -> SBUF bf16 (the pace-setter:
            32 x [128,1024] = ~35.6us busy)
  mask      DVE multiply by keep (=~mask) bf16, 2x_1p mode (~0.7us/unit)
  ctx       bf16 matmul, v65 = [v | 1] stationary (row 64 = softmax denom)
  epilogue  PE transposes + reciprocal + scale, interleaved into pass 1

Loop is q-half-major (2 passes x 16 k-tiles) so ctx PSUM is 2 banks and the
score PSUM triple-buffers (3x2 banks): 3*2 + 2 = 8 banks, keeping ScalarE
gap-free. v projections are interleaved into pass 0 so their LDWEIGHTS hide
under main-loop matmuls.
"""

import math
import sys
from contextlib import ExitStack

import numpy as np

sys.path.insert(0, "/opt/trn_rl_repo")

import ml_dtypes  # noqa: E402

import concourse.bass as bass  # noqa: E402
import concourse.tile as tile  # noqa: E402
from concourse import bacc, mybir  # noqa: E402
from concourse.bass import ds  # noqa: E402
from concourse.bass_utils import run_bass_kernel_spmd  # noqa: E402
from concourse.masks import make_identity  # noqa: E402

B, S, DIN, DOUT = 8, 2048, 768, 64
P = 128
NCH = 6  # din chunks
KT = S // P  # 16 key tiles
NB = 4  # qk projection column blocks of 512
NS = 512  # matmul moving free dim (one PSUM bank fp32)
H = 2  # q halves (passes)
HQ = S // H  # 1024

F32 = mybir.dt.float32
BF16 = mybir.dt.bfloat16

_NC_CACHE = None


def build_nc():
    nc = bacc.Bacc("TRN2", target_bir_lowering=False, debug=False)

    xT = nc.declare_dram_parameter("xT", [NCH, P, S], BF16, isOutput=False)
    keep = nc.declare_dram_parameter("keep", [KT, P, S], BF16, isOutput=False)
    wqk = nc.declare_dram_parameter("wqk", [NCH, P, P], BF16, isOutput=False)
    wv = nc.declare_dram_parameter("wv", [NCH, P, DOUT], BF16, isOutput=False)
    bqk = nc.declare_dram_parameter("bqk", [P, 1], F32, isOutput=False)
    out = nc.declare_dram_parameter("out", [S, DOUT], F32, isOutput=True)

    inv_sqrt_s = float(1.0 / math.sqrt(S))

    with tile.TileContext(nc) as tc, ExitStack() as ctx:
        singles = ctx.enter_context(tc.tile_pool(name="singles", bufs=1))
        epool = ctx.enter_context(tc.tile_pool(name="epool", bufs=3))
        opool = ctx.enter_context(tc.tile_pool(name="opool", bufs=4))

        # ---- constants / weights (small DMAs first)
        wqk_sb = singles.tile([P, NCH, P], BF16)
        nc.sync.dma_start(out=wqk_sb, in_=wqk.rearrange("c p m -> p c m"))
        wv_sb = singles.tile([P, NCH, DOUT], BF16)
        nc.sync.dma_start(out=wv_sb, in_=wv.rearrange("c p m -> p c m"))
        bqk_sb = singles.tile([P, 1], F32)
        nc.sync.dma_start(out=bqk_sb, in_=bqk[:, :])

        # ---- big inputs, in consumption-priority order:
        # x col-block 0 (all chunks) -> keep t0 -> x block 1 -> keep t1 ->
        # x blocks 2,3 -> keep t2..15
        xT_sb = singles.tile([P, NCH, S], BF16)
        keep_sb = singles.tile([P, KT, S], BF16)
        for c in range(NCH):
            nc.sync.dma_start(out=xT_sb[:, c, ds(0, NS)], in_=xT[c, :, ds(0, NS)])
        nc.sync.dma_start(out=keep_sb[:, 0, :], in_=keep[0, :, :])
        for c in range(NCH):
            nc.sync.dma_start(out=xT_sb[:, c, ds(NS, NS)], in_=xT[c, :, ds(NS, NS)])
        nc.sync.dma_start(out=keep_sb[:, 1, :], in_=keep[1, :, :])
        for blk in range(2, NB):
            for c in range(NCH):
                nc.sync.dma_start(
                    out=xT_sb[:, c, ds(blk * NS, NS)], in_=xT[c, :, ds(blk * NS, NS)]
                )
        for t in range(2, KT):
            nc.sync.dma_start(out=keep_sb[:, t, :], in_=keep[t, :, :])

        ident = singles.tile([P, P], F32)
        make_identity(nc, ident)

        # ---- v with a ones column: [s(128 part), ktile, 65] bf16
        v65_sb = singles.tile([P, KT, DOUT + 1], BF16)
        nc.gpsimd.memset(v65_sb, 1.0)

        warm_sb = singles.tile([P, NS], BF16)
        nc.gpsimd.memset(warm_sb, 0.0)

        qT_sb = singles.tile([DOUT, S], BF16)
        kT_sb = singles.tile([DOUT, S], BF16)
        ctxT_sb = singles.tile([DOUT + 1, S], F32)

        with (
            tc.tile_pool(name="psS", bufs=3, space="PSUM") as psS,
            tc.tile_pool(name="psC", bufs=1, space="PSUM") as psC,
        ):

            def filler(n):
                # prologue-only scratch matmuls to hold the HAM gate at 8/8
                wps = psS.tile([P, HQ], F32, tag="big", name="wps")
                for _ in range(n):
                    nc.tensor.matmul(
                        wps[:, 0:NS], lhsT=warm_sb[:, 0:P], rhs=warm_sb[:, 0:NS],
                        start=True, stop=True, skip_group_check=True,
                    )

            def vproj(t):
                v_ps = psS.tile([P, HQ], F32, tag="big")
                for c in range(NCH):
                    nc.tensor.matmul(
                        v_ps[:, 0:DOUT],
                        lhsT=xT_sb[:, c, ds(t * P, P)],
                        rhs=wv_sb[:, c, :],
                        start=(c == 0),
                        stop=(c == NCH - 1),
                    )
                nc.vector.tensor_copy(v65_sb[:, t, 0:DOUT], v_ps[:, 0:DOUT])

            # ---- HAM warmup while the first x blocks stream in
            filler(10)

            # ---- qk projection + split into qT/kT (bias added on DVE)
            for blk in range(NB):
                qk_ps = psS.tile([P, HQ], F32, tag="big")
                for c in range(NCH):
                    nc.tensor.matmul(
                        qk_ps[:, 0:NS],
                        lhsT=wqk_sb[:, c, :],
                        rhs=xT_sb[:, c, ds(blk * NS, NS)],
                        start=(c == 0),
                        stop=(c == NCH - 1),
                    )
                cols = ds(blk * NS, NS)
                nc.vector.tensor_scalar_add(
                    qT_sb[:, cols], qk_ps[0:DOUT, 0:NS], bqk_sb[0:DOUT]
                )
                nc.vector.tensor_scalar_add(
                    kT_sb[:, cols], qk_ps[DOUT:P, 0:NS], bqk_sb[DOUT:P]
                )
                if blk == 1:
                    vproj(0)
                filler(4)

            filler(8)

            # ---- epilogue worker: transpose back, normalize, stage stores
            ostage = {}

            def epilogue_tile(h, qt):
                tr = psS.tile([P, HQ], F32, tag="big")
                nc.tensor.transpose(
                    tr[:, 0 : DOUT + 1],
                    ctxT_sb[:, ds(h * HQ + qt * P, P)],
                    ident[0 : DOUT + 1, 0 : DOUT + 1],
                )
                rc = opool.tile([P, 1], F32, tag="rc")
                nc.vector.reciprocal(rc, tr[:, DOUT : DOUT + 1])
                g, gi = qt // 4, qt % 4
                if gi == 0:
                    ostage[(h, g)] = opool.tile(
                        [P, 4, DOUT], F32, tag="ostage", name=f"ostage_{h}_{g}"
                    )
                nc.vector.tensor_scalar_mul(
                    ostage[(h, g)][:, gi, :], tr[:, 0:DOUT], rc
                )
                if gi == 3:
                    nc.sync.dma_start(
                        out=out[ds(h * HQ + g * 4 * P, 4 * P), :].rearrange(
                            "(t p) m -> p t m", p=P
                        ),
                        in_=ostage.pop((h, g)),
                    )

            # ---- main loop: q-half-major, 16 k-tiles inside
            ep_queue = []  # deferred epilogue work, interleaved into pass 1
            for h in range(H):
                ctx_ps = psC.tile([DOUT + 1, HQ], F32)
                for t in range(KT):
                    sc = psS.tile([P, HQ], F32, tag="big")
                    for n in range(HQ // NS):
                        nc.tensor.matmul(
                            sc[:, ds(n * NS, NS)],
                            lhsT=kT_sb[:, ds(t * P, P)],
                            rhs=qT_sb[:, ds(h * HQ + n * NS, NS)],
                            start=True,
                            stop=True,
                        )
                    ex = epool.tile([P, HQ], BF16, tag="exp")
                    nc.scalar.activation(
                        out=ex,
                        in_=sc,
                        func=mybir.ActivationFunctionType.Exp,
                        scale=inv_sqrt_s,
                    )
                    nc.vector.tensor_mul(ex, ex, keep_sb[:, t, ds(h * HQ, HQ)])
                    for n in range(HQ // NS):
                        nc.tensor.matmul(
                            ctx_ps[:, ds(n * NS, NS)],
                            lhsT=v65_sb[:, t, :],
                            rhs=ex[:, ds(n * NS, NS)],
                            start=(t == 0),
                            stop=(t == KT - 1),
                        )
                    if h == 0 and t < KT - 1:
                        vproj(t + 1)
                    # spread pass-0 epilogue work into pass 1
                    if ep_queue and t >= 1:
                        epilogue_tile(*ep_queue.pop(0))

                nc.vector.tensor_copy(ctxT_sb[:, ds(h * HQ, HQ)], ctx_ps)
                ep_queue.extend((h, qt) for qt in range(HQ // P))

            while ep_queue:
                epilogue_tile(*ep_queue.pop(0))

    nc.finalize()
    return nc


def _get_nc():
    global _NC_CACHE
    if _NC_CACHE is None:
        _NC_CACHE = build_nc()
    return _NC_CACHE


def kernel(**inputs):
    x = np.asarray(inputs["input_tensor"], dtype=np.float32)  # [B, S, DIN]
    mask = np.asarray(inputs["attention_mask"])  # [B, S, S] bool
    Wq = np.asarray(inputs["Wq"], dtype=np.float32)
    Wk = np.asarray(inputs["Wk"], dtype=np.float32)
    Wv = np.asarray(inputs["Wv"], dtype=np.float32)
    bq = np.asarray(inputs["bq"], dtype=np.float32)
    bk = np.asarray(inputs["bk"], dtype=np.float32)
    bv = np.asarray(inputs["bv"], dtype=np.float32)

    Wqk = np.concatenate([Wq, Wk], axis=1)  # [768, 128]
    wqk_h = np.ascontiguousarray(Wqk.reshape(NCH, P, P)).astype(ml_dtypes.bfloat16)
    wv_h = np.ascontiguousarray(Wv.reshape(NCH, P, DOUT)).astype(ml_dtypes.bfloat16)
    bqk_h = np.ascontiguousarray(np.concatenate([bq, bk]).reshape(P, 1))

    in_maps = []
    for b in range(B):
        xTb = np.ascontiguousarray(x[b].T)  # [DIN, S] fp32
        xT_h = np.ascontiguousarray(xTb.reshape(NCH, P, S)).astype(ml_dtypes.bfloat16)
        # keep = ~mask, transposed to [k, q], per key tile
        keepT = (~mask[b]).T
        keep_h = np.ascontiguousarray(keepT.reshape(KT, P, S)).astype(
            ml_dtypes.bfloat16
        )
        in_maps.append(
            {
                "xT": xT_h,
                "keep": keep_h,
                "wqk": wqk_h,
                "wv": wv_h,
                "bqk": bqk_h,
            }
        )

    nc = _get_nc()
    res = run_bass_kernel_spmd(nc, in_maps, core_ids=list(range(B)))
    out = np.stack([np.asarray(res.results[b]["out"], np.float32) for b in range(B)])
    out = out + bv[None, None, :]
    return out.astype(np.float32)


# revision 10
# speedup vs baseline: 1.9660x; 1.1617x over previous
"""Trainium2 Bass kernel for single-head attention (B=8, S=2048, DIN=768, DOUT=64).

Strategy: data parallel — one batch element per NeuronCore (8 cores).
Per core, attention runs in transposed-score layout (k on partitions, q on
free dim), ScalarE-paced at ~1.11us per [128,1024] exp. All math is bf16
with fp32 PSUM accumulation (fp8/DoubleRow measures 2 cyc/col on this HW —
no gain — so bf16 at 1 col/cycle is optimal).

  warmup    scratch matmuls bridge every prologue gap so the PE HAM clock
            gate latches 8/8 (2.4 GHz) and the main loop never idles long
            enough (>3.4us) to drop back to 1.2 GHz
  qk proj   [Wq|Wk] combined: 6 chunk matmuls per 512-col block ->
            PSUM [q|k, cols]; DVE splits into qT/kT (bias added)
  scores    kT-tile stationary [64,128], qT moving -> sc[k,q] PSUM
  exp       ScalarE activation, PSUM fp32 -> SBUF bf16 (the pace-setter:
            32 x [128,1024] = ~35.6us busy)
  mask      DVE multiply by keep (=~mask) bf16, 2x_1p mode (~0.7us/unit)
  ctx       bf16 matmul, v65 = [v | 1] stationary (row 64 = softmax denom)
  epilogue  PE transposes + reciprocal + scale, interleaved into pass 1

Loop is q-half-major (2 passes x 16 k-tiles) so ctx PSUM is 2 banks and the
score PSUM triple-buffers (3x2 banks): 3*2 + 2 = 8 banks, keeping ScalarE
gap-free. v projections are interleaved into pass 0 so their LDWEIGHTS hide
under main-loop matmuls.
"""

import math
import sys
from contextlib import ExitStack

import numpy as np

sys.path.insert(0, "/opt/trn_rl_repo")

import ml_dtypes  # noqa: E402

import concourse.bass as bass  # noqa: E402
import concourse.tile as tile  # noqa: E402
from concourse import bacc, mybir  # noqa: E402
from concourse.bass import ds  # noqa: E402
from concourse.bass_utils import run_bass_kernel_spmd  # noqa: E402
from concourse.masks import make_identity  # noqa: E402

B, S, DIN, DOUT = 8, 2048, 768, 64
P = 128
NCH = 6  # din chunks
KT = S // P  # 16 key tiles
NB = 4  # qk projection column blocks of 512
NS = 512  # matmul moving free dim (one PSUM bank fp32)
H = 2  # q halves (passes)
HQ = S // H  # 1024

F32 = mybir.dt.float32
BF16 = mybir.dt.bfloat16

_NC_CACHE = None


def build_nc():
    nc = bacc.Bacc("TRN2", target_bir_lowering=False, debug=False)

    xT = nc.declare_dram_parameter("xT", [NCH, P, S], BF16, isOutput=False)
    keep = nc.declare_dram_parameter("keep", [KT, P, S], BF16, isOutput=False)
    wqk = nc.declare_dram_parameter("wqk", [NCH, P, P], BF16, isOutput=False)
    wv = nc.declare_dram_parameter("wv", [NCH, P, DOUT], BF16, isOutput=False)
    bqk = nc.declare_dram_parameter("bqk", [P, 1], F32, isOutput=False)
    out = nc.declare_dram_parameter("out", [S, DOUT], F32, isOutput=True)

    inv_sqrt_s = float(1.0 / math.sqrt(S))

    with tile.TileContext(nc) as tc, ExitStack() as ctx:
        singles = ctx.enter_context(tc.tile_pool(name="singles", bufs=1))
        epool = ctx.enter_context(tc.tile_pool(name="epool", bufs=3))
        opool = ctx.enter_context(tc.tile_pool(name="opool", bufs=4))

        # ---- constants / weights (small DMAs first)
        wqk_sb = singles.tile([P, NCH, P], BF16)
        nc.sync.dma_start(out=wqk_sb, in_=wqk.rearrange("c p m -> p c m"))
        wv_sb = singles.tile([P, NCH, DOUT], BF16)
        nc.sync.dma_start(out=wv_sb, in_=wv.rearrange("c p m -> p c m"))
        bqk_sb = singles.tile([P, 1], F32)
        nc.sync.dma_start(out=bqk_sb, in_=bqk[:, :])

        # ---- big inputs, in consumption-priority order: x blocks 0-1
        # (gate the first exp), keep pass-0 halves for early tiles, x blocks
        # 2-3, the rest of keep pass-0, then all keep pass-1 halves.
        xT_sb = singles.tile([P, NCH, S], BF16)
        keep_sb = singles.tile([P, KT, S], BF16)

        def dma_x_block(blk):
            for c in range(NCH):
                nc.sync.dma_start(
                    out=xT_sb[:, c, ds(blk * NS, NS)], in_=xT[c, :, ds(blk * NS, NS)]
                )

        def dma_keep_half(t, h):
            nc.sync.dma_start(
                out=keep_sb[:, t, ds(h * HQ, HQ)], in_=keep[t, :, ds(h * HQ, HQ)]
            )

        dma_x_block(0)
        dma_x_block(1)
        for t in range(4):
            dma_keep_half(t, 0)
        dma_x_block(2)
        dma_x_block(3)
        for t in range(4, KT):
            dma_keep_half(t, 0)
        for t in range(KT):
            dma_keep_half(t, 1)

        ident = singles.tile([P, P], F32)
        make_identity(nc, ident)

        # ---- v with a ones column: [s(128 part), ktile, 65] bf16
        v65_sb = singles.tile([P, KT, DOUT + 1], BF16)
        nc.gpsimd.memset(v65_sb, 1.0)

        warm_sb = singles.tile([P, NS], BF16)
        nc.gpsimd.memset(warm_sb, 0.0)

        qT_sb = singles.tile([DOUT, S], BF16)
        kT_sb = singles.tile([DOUT, S], BF16)
        ctxT_sb = singles.tile([DOUT + 1, S], F32)

        with (
            tc.tile_pool(name="psS", bufs=3, space="PSUM") as psS,
            tc.tile_pool(name="psC", bufs=1, space="PSUM") as psC,
        ):

            def filler(n):
                # prologue-only scratch matmuls to hold the HAM gate at 8/8
                wps = psS.tile([P, HQ], F32, tag="big", name="wps")
                for _ in range(n):
                    nc.tensor.matmul(
                        wps[:, 0:NS], lhsT=warm_sb[:, 0:P], rhs=warm_sb[:, 0:NS],
                        start=True, stop=True, skip_group_check=True,
                    )

            def vproj(t):
                v_ps = psS.tile([P, HQ], F32, tag="big")
                for c in range(NCH):
                    nc.tensor.matmul(
                        v_ps[:, 0:DOUT],
                        lhsT=xT_sb[:, c, ds(t * P, P)],
                        rhs=wv_sb[:, c, :],
                        start=(c == 0),
                        stop=(c == NCH - 1),
                    )
                nc.vector.tensor_copy(v65_sb[:, t, 0:DOUT], v_ps[:, 0:DOUT])

            # ---- HAM warmup while the first x blocks stream in
            filler(10)

            # ---- qk projection + split into qT/kT (bias added on DVE)
            for blk in range(NB):
                qk_ps = psS.tile([P, HQ], F32, tag="big")
                for c in range(NCH):
                    nc.tensor.matmul(
                        qk_ps[:, 0:NS],
                        lhsT=wqk_sb[:, c, :],
                        rhs=xT_sb[:, c, ds(blk * NS, NS)],
                        start=(c == 0),
                        stop=(c == NCH - 1),
                    )
                cols = ds(blk * NS, NS)
                nc.vector.tensor_scalar_add(
                    qT_sb[:, cols], qk_ps[0:DOUT, 0:NS], bqk_sb[0:DOUT]
                )
                nc.vector.tensor_scalar_add(
                    kT_sb[:, cols], qk_ps[DOUT:P, 0:NS], bqk_sb[DOUT:P]
                )
                if blk == 1:
                    vproj(0)
                filler(4)

            filler(8)

            # ---- epilogue worker: transpose back, normalize, stage stores
            ostage = {}

            def epilogue_tile(h, qt):
                tr = psS.tile([P, HQ], F32, tag="big")
                nc.tensor.transpose(
                    tr[:, 0 : DOUT + 1],
                    ctxT_sb[:, ds(h * HQ + qt * P, P)],
                    ident[0 : DOUT + 1, 0 : DOUT + 1],
                )
                rc = opool.tile([P, 1], F32, tag="rc")
                nc.vector.reciprocal(rc, tr[:, DOUT : DOUT + 1])
                g, gi = qt // 4, qt % 4
                if gi == 0:
                    ostage[(h, g)] = opool.tile(
                        [P, 4, DOUT], F32, tag="ostage", name=f"ostage_{h}_{g}"
                    )
                nc.vector.tensor_scalar_mul(
                    ostage[(h, g)][:, gi, :], tr[:, 0:DOUT], rc
                )
                if gi == 3:
                    nc.sync.dma_start(
                        out=out[ds(h * HQ + g * 4 * P, 4 * P), :].rearrange(
                            "(t p) m -> p t m", p=P
                        ),
                        in_=ostage.pop((h, g)),
                    )

            # ---- main loop: q-half-major, 16 k-tiles inside
            for h in range(H):
                ctx_ps = psC.tile([DOUT + 1, HQ], F32)
                for t in range(KT):
                    sc = psS.tile([P, HQ], F32, tag="big")
                    for n in range(HQ // NS):
                        nc.tensor.matmul(
                            sc[:, ds(n * NS, NS)],
                            lhsT=kT_sb[:, ds(t * P, P)],
                            rhs=qT_sb[:, ds(h * HQ + n * NS, NS)],
                            start=True,
                            stop=True,
                        )
                    ex = epool.tile([P, HQ], BF16, tag="exp")
                    nc.scalar.activation(
                        out=ex,
                        in_=sc,
                        func=mybir.ActivationFunctionType.Exp,
                        scale=inv_sqrt_s,
                    )
                    nc.vector.tensor_mul(ex, ex, keep_sb[:, t, ds(h * HQ, HQ)])
                    for n in range(HQ // NS):
                        nc.tensor.matmul(
                            ctx_ps[:, ds(n * NS, NS)],
                            lhsT=v65_sb[:, t, :],
                            rhs=ex[:, ds(n * NS, NS)],
                            start=(t == 0),
                            stop=(t == KT - 1),
                        )
                    if h == 0 and t < KT - 1:
                        vproj(t + 1)

                nc.vector.tensor_copy(ctxT_sb[:, ds(h * HQ, HQ)], ctx_ps)
                if h == 0:
                    filler(4)

            for h in range(H):
                for qt in range(HQ // P):
                    epilogue_tile(h, qt)

    nc.finalize()
    return nc


def _get_nc():
    global _NC_CACHE
    if _NC_CACHE is None:
        _NC_CACHE = build_nc()
    return _NC_CACHE


def kernel(**inputs):
    x = np.asarray(inputs["input_tensor"], dtype=np.float32)  # [B, S, DIN]
    mask = np.asarray(inputs["attention_mask"])  # [B, S, S] bool
    Wq = np.asarray(inputs["Wq"], dtype=np.float32)
    Wk = np.asarray(inputs["Wk"], dtype=np.float32)
    Wv = np.asarray(inputs["Wv"], dtype=np.float32)
    bq = np.asarray(inputs["bq"], dtype=np.float32)
    bk = np.asarray(inputs["bk"], dtype=np.float32)
    bv = np.asarray(inputs["bv"], dtype=np.float32)

    Wqk = np.concatenate([Wq, Wk], axis=1)  # [768, 128]
    wqk_h = np.ascontiguousarray(Wqk.reshape(NCH, P, P)).astype(ml_dtypes.bfloat16)
    wv_h = np.ascontiguousarray(Wv.reshape(NCH, P, DOUT)).astype(ml_dtypes.bfloat16)
    bqk_h = np.ascontiguousarray(np.concatenate([bq, bk]).reshape(P, 1))

    in_maps = []
    for b in range(B):
        xTb = np.ascontiguousarray(x[b].T)  # [DIN, S] fp32
        xT_h = np.ascontiguousarray(xTb.reshape(NCH, P, S)).astype(ml_dtypes.bfloat16)
        # keep = ~mask, transposed to [k, q], per key tile
        keepT = (~mask[b]).T
        keep_h = np.ascontiguousarray(keepT.reshape(KT, P, S)).astype(
            ml_dtypes.bfloat16
        )
        in_maps.append(
            {
                "xT": xT_h,
                "keep": keep_h,
                "wqk": wqk_h,
                "wv": wv_h,
                "bqk": bqk_h,
            }
        )

    nc = _get_nc()
    res = run_bass_kernel_spmd(nc, in_maps, core_ids=list(range(B)))
    out = np.stack([np.asarray(res.results[b]["out"], np.float32) for b in range(B)])
    out = out + bv[None, None, :]
    return out.astype(np.float32)
